# revision 3
# baseline (speedup 1.0000x reference)
"""LoRA embedding lookup on 8 Trainium2 NeuronCores.

out = weight[ids] + ((lora_B @ lora_A).T * 2.0)[ids]
    = weight[ids] + (lora_A.T[ids] @ (lora_B * 2.0).T)

Strategy: token-parallel. Each of the 8 cores owns 2048 of the 16384
tokens. Host prep concatenates [weight | lora_A.T] into one table so a
single indirect-DMA gather per 128-token tile fetches both the base
embedding row and the 8 LoRA-A coefficients. On-core, the coefficients
are PE-transposed and hit with a tiny [8,128]x[8,1024] matmul against
(lora_B*2).T, accumulated onto the base rows by VectorE, and stored.
No collectives needed.
"""

import numpy as np

import concourse.bacc as bacc
import concourse.bass as bass
import concourse.mybir as mybir
import concourse.tile as tile
from concourse.bass_utils import run_bass_kernel_spmd
from concourse.masks import make_identity

VOCAB = 128000
D = 1024
R = 8
SCALING = 2.0
N_CORES = 8
P = 128
CHUNK = 512  # matmul free-dim / PSUM bank size in f32

# test.py can inject extra kwargs (e.g. trace=True) and read back results
_RUN_KWARGS: dict = {}
LAST_RESULT = None


def build_nc(vocab: int, d: int, r: int, ntiles: int, repeat: int = 1):
    """Per-core SPMD graph: gather+LoRA for ntiles*128 tokens.

    repeat>1 re-runs the whole pipeline (same ids, same outputs) for
    within-NEFF timing amplification; results are unchanged.
    """
    dw = d + r
    nc = bacc.Bacc(None, target_bir_lowering=False, debug=False)

    wcat = nc.dram_tensor("wcat", [vocab, dw], mybir.dt.float32, kind="ExternalInput")
    bst = nc.dram_tensor("bst", [r, d], mybir.dt.float32, kind="ExternalInput")
    ids = nc.dram_tensor("ids", [P, ntiles], mybir.dt.int32, kind="ExternalInput")
    out = nc.dram_tensor("out", [ntiles * P, d], mybir.dt.float32, kind="ExternalOutput")

    with tile.TileContext(nc) as tc:
        with (
            tc.tile_pool(name="const", bufs=1) as const_pool,
            tc.tile_pool(name="work", bufs=4) as work_pool,
            tc.tile_pool(name="small", bufs=3) as small_pool,
            tc.tile_pool(name="psum_mm", bufs=4, space="PSUM") as psum_mm,
            tc.tile_pool(name="psum_tr", bufs=2, space="PSUM") as psum_tr,
        ):
            ids_tile = const_pool.tile([P, ntiles], mybir.dt.int32)
            nc.sync.dma_start(out=ids_tile[:], in_=ids[:])
            bst_tile = const_pool.tile([r, d], mybir.dt.float32)
            nc.sync.dma_start(out=bst_tile[:], in_=bst[:])
            ident = const_pool.tile([P, P], mybir.dt.float32)
            make_identity(nc, ident[:])

            for i in [t for _ in range(repeat) for t in range(ntiles)]:
                gtile = work_pool.tile([P, dw], mybir.dt.float32, tag="g")
                nc.gpsimd.indirect_dma_start(
                    out=gtile[:],
                    out_offset=None,
                    in_=wcat[:],
                    in_offset=bass.IndirectOffsetOnAxis(
                        ap=ids_tile[:, i : i + 1], axis=0
                    ),
                )
                # a-coeffs [128, r] -> [r, 128] via PE transpose
                atp = psum_tr.tile([P, P], mybir.dt.float32, tag="atp")
                nc.tensor.transpose(
                    out=atp[:r, :], in_=gtile[:, d:dw], identity=ident[:]
                )
                at_sb = small_pool.tile([r, P], mybir.dt.float32, tag="at")
                nc.vector.tensor_copy(at_sb[:], atp[:r, :])
                for h in range(0, d, CHUNK):
                    dp = psum_mm.tile([P, CHUNK], mybir.dt.float32, tag="dp")
                    nc.tensor.matmul(
                        dp[:],
                        at_sb[:],
                        bst_tile[:, h : h + CHUNK],
                        start=True,
                        stop=True,
                    )
                    nc.vector.tensor_add(
                        out=gtile[:, h : h + CHUNK],
                        in0=gtile[:, h : h + CHUNK],
                        in1=dp[:],
                    )
                nc.sync.dma_start(
                    out=out[i * P : (i + 1) * P, :], in_=gtile[:, :d]
                )

    nc.compile()
    return nc


def _prep_inputs(input_ids, weight, lora_A, lora_B):
    ids = np.ascontiguousarray(np.asarray(input_ids).reshape(-1).astype(np.int32))
    w = np.asarray(weight, dtype=np.float32)
    a_t = np.asarray(lora_A, dtype=np.float32).T  # [vocab, r]
    wcat = np.ascontiguousarray(np.concatenate([w, a_t], axis=1))
    bst = np.ascontiguousarray(np.asarray(lora_B, dtype=np.float32).T * SCALING)
    return ids, wcat, bst


def kernel(input_ids, weight, lora_A, lora_B):
    global LAST_RESULT
    ids, wcat, bst = _prep_inputs(input_ids, weight, lora_A, lora_B)
    ntok = ids.size
    assert ntok % (N_CORES * P) == 0
    tpc = ntok // N_CORES
    ntiles = tpc // P

    nc = build_nc(wcat.shape[0], D, R, ntiles)

    in_maps = []
    for c in range(N_CORES):
        ids_c = ids[c * tpc : (c + 1) * tpc].reshape(ntiles, P).T
        in_maps.append(
            {"wcat": wcat, "bst": bst, "ids": np.ascontiguousarray(ids_c)}
        )

    res = run_bass_kernel_spmd(nc, in_maps, list(range(N_CORES)), **_RUN_KWARGS)
    LAST_RESULT = res
    outs = [res.results[c]["out"] for c in range(N_CORES)]
    full = np.concatenate(outs, axis=0)
    return full.reshape(*np.asarray(input_ids).shape, D).astype(np.float32)
